# revision 1
# baseline (speedup 1.0000x reference)
"""MiniMaxText01 linear attention layer on 8 Trainium2 NeuronCores.

Tensor-parallel over heads (4 heads per core). Per core:
  - fused QKV+gate projection computed in transposed layout
    (features on partitions, sequence on free dim) with bf16 matmuls;
    gate sigmoid computed as 0.5*(1+tanh(x/2)) so every ACT function
    (Silu/Tanh/Square/Copy) lives in one LUT table set
  - lightning (chunked linear) attention with per-head decay, two heads
    packed per 128-partition group (PE row/col tiling)
  - RMSNorm variance: ones-matmul partition reduction, per-chunk 2KB
    AllReduce across the 8 cores, DVE-only Newton rsqrt; out-projection
    for chunk c runs one chunk behind the attention pipeline so the
    collective latency is hidden
  - out-proj row-parallel: each core emits a full-width partial output
    (transposed, f16 to halve the PSUM->SBUF copy + store cost); host
    sums the 8 partials and transposes back.
  - attention matmul count reduced vs the naive packing: the inter
    (carry-state) term uses one matmul per head-pair against a
    block-diagonal S image, and the KV state update uses full-width
    128x128 products whose off-diagonal head quadrants are ignored.

Everything is hardcoded for the fixed problem shapes below.
"""

import math
import warnings

warnings.filterwarnings("ignore")

import numpy as np
import ml_dtypes

import concourse.bacc as bacc
import concourse.mybir as mybir
import concourse.tile as tile
from concourse.bass_utils import run_bass_kernel_spmd
from concourse.masks import make_identity

F32 = mybir.dt.float32
I32 = mybir.dt.int32
BF16 = mybir.dt.bfloat16
BF = ml_dtypes.bfloat16
AF = mybir.ActivationFunctionType
ALU = mybir.AluOpType

N = 8192          # sequence length
HID = 2048        # hidden size
H = 32            # total heads
D = 64            # head dim
BLOCK = 256       # attention chunk size
NCORES = 8
HL = H // NCORES  # 4 local heads per core
CHUNK = 512       # seq columns processed per projection chunk
NCHUNK = N // CHUNK
BPC = CHUNK // BLOCK  # blocks per chunk
EPS = 1e-5
NUM_LAYERS, LAYER_IDX = 80, 0
MAGIC = 0x5F3759DF

LAST_EXEC_NS = None
LAST_RESULTS = None


def _build_slopes(n):
    def p2(m):
        start = 2 ** (-(2 ** (-(math.log2(m) - 3))))
        return [start * start**i for i in range(m)]

    if math.log2(n).is_integer():
        s = p2(n)
    else:
        cp = 2 ** math.floor(math.log2(n))
        s = p2(cp) + _build_slopes(2 * cp).tolist()[0::2][: n - cp]
    return np.array(s, dtype=np.float32)


SLOPE = _build_slopes(H) * (1.0 - LAYER_IDX / (NUM_LAYERS - 1) + 1e-5)  # [H]

_NC_CACHE = None


DEFAULT_OPTS = dict(
    psA=3, psB=2, psC=2, psV=1,
    hstb=3, combb=2, gateb=2, ghb=7, kvb=4, kptb=4, qpb=3, qkpb=3, hsqb=3, osbb=8,
    lag=5, taper=0, cc_stride=1, rbcb=2, rcpb=2,
    no_cc=False,  # replace the AllReduce with a local copy (TimelineSim only)
    interbd=0,    # inter-term via one block-diagonal-S matmul per p
    statemerge=0, # state update via full-width 128x128 matmuls
    hst_contig=0, # host-side chunk-major hsT layout -> contiguous chunk loads
    rsq_flat=0,   # rsqrt DVE math on [1, RW] (one descriptor) instead of [128, RW/128]
    osb_batch=0,  # out-proj stores batched 4 mt-tiles per DMA
    tpcomb=0,     # combine k/v transposes into one PSUM tile per p
    osb_f16=0,    # out-proj partials stored + output in f16
    pooloff=0,    # offload SBUF-only elementwise muls to GpSimd
    ilv=0,        # interleave next chunk's projection slices between attention phases
    # timing-only ablations (wrong results):
    no_attn=0, no_outproj=0, dma_only=0, proj_noload=0,
    rep=1,        # repeat the whole body N times in one NEFF (device-time probe)
    small_out=0,  # same device work, outp redirected to DRAM scratch (tiny output)
    small_in=0,   # same device work, hsT read from DRAM scratch (tiny input)
)


def _build_module(**opts):
    o = dict(DEFAULT_OPTS)
    o.update(opts)
    nc = bacc.Bacc("TRN2", target_bir_lowering=False, num_devices=NCORES)

    hst_shape = (
        [128, NCHUNK, HID // 128, CHUNK] if o["hst_contig"] else [HID, N]
    )
    if o["small_in"]:
        hsT_d = nc.dram_tensor("hsT", [128, 128], BF16, kind="ExternalInput")
    else:
        hsT_d = nc.dram_tensor("hsT", hst_shape, BF16, kind="ExternalInput")
    wc_d = nc.dram_tensor("wcomb", [8, 128, HID], BF16, kind="ExternalInput")
    wo_d = nc.dram_tensor("wout", [2 * 128, HID], BF16, kind="ExternalInput")
    dd_d = nc.dram_tensor("dd", [128, 2 * HL, BLOCK], F32, kind="ExternalInput")
    qd_d = nc.dram_tensor("qd", [128, HL // 2, BLOCK], BF16, kind="ExternalInput")
    kdb_d = nc.dram_tensor("kdb", [128, HL // 2, BLOCK], BF16, kind="ExternalInput")
    bd_d = nc.dram_tensor("bd", [128, HL // 2], F32, kind="ExternalInput")
    kv0_d = nc.dram_tensor("kv0", [128, HL // 2, D], F32, kind="ExternalInput")
    out_dt = mybir.dt.float16 if o["osb_f16"] else F32
    if o["small_out"]:
        outp_d = nc.dram_tensor("outp", [128, 128], out_dt, kind="ExternalOutput")
    else:
        outp_d = nc.dram_tensor("outp", [HID, N], out_dt, kind="ExternalOutput")

    with tile.TileContext(nc) as tc:
        with (
            tc.tile_pool(name="singles", bufs=1) as sg,
            tc.tile_pool(name="hstp", bufs=o["hstb"]) as hstp,
            tc.tile_pool(name="combp", bufs=o["combb"]) as combp,
            tc.tile_pool(name="gatep", bufs=o["gateb"]) as gatep,
            tc.tile_pool(name="ghp", bufs=o["ghb"]) as ghp,
            tc.tile_pool(name="kvp", bufs=o["kvb"]) as kvp,
            tc.tile_pool(name="kptp", bufs=o["kptb"]) as kptp,
            tc.tile_pool(name="qpp", bufs=o["qpb"]) as qpp,
            tc.tile_pool(name="qkpp", bufs=o["qkpb"]) as qkpp,
            tc.tile_pool(name="hsqp", bufs=o["hsqb"]) as hsqp,
            tc.tile_pool(name="osbp", bufs=o["osbb"]) as osbp,
            tc.tile_pool(name="rcp", bufs=o["rcpb"]) as rcp,
            tc.tile_pool(name="psA", bufs=o["psA"], space="PSUM") as psA,
            tc.tile_pool(name="psB", bufs=o["psB"], space="PSUM") as psB,
            tc.tile_pool(name="psC", bufs=o["psC"], space="PSUM") as psC,
            tc.tile_pool(name="psV", bufs=o["psV"], space="PSUM") as psV,
            tc.tile_pool(name="dram", bufs=1, space="DRAM") as dram,
        ):
            # ---- resident tensors -------------------------------------
            wcm = []
            for mt in range(8):
                wct = sg.tile([128, HID // 128, 128], BF16, name=f"wcm{mt}")
                wcm.append(wct)
            wo_sb = sg.tile([128, 2, HID], BF16)
            dd_sb = sg.tile([128, 2 * HL, BLOCK], F32)
            qd_sb = sg.tile([128, HL // 2, BLOCK], BF16)
            kdb_sb = sg.tile([128, HL // 2, BLOCK], BF16)
            bd_sb = sg.tile([128, HL // 2], F32)
            S32 = sg.tile([128, HL // 2, D], F32)
            if o["interbd"]:
                # block-diagonal S: head hi's [64,64] state sits at rows
                # hi*64, cols hi*64; off-diagonal quadrants stay zero forever
                Sbd = sg.tile([128, HL // 2, 128], BF16)
                nc.vector.memset(Sbd, 0.0)
            else:
                Sbf = sg.tile([128, HL // 2, D], BF16)
            ones_sb = sg.tile([128, 1], BF16)
            nc.vector.memset(ones_sb, 1.0)
            ident = sg.tile([128, 128], BF16)
            make_identity(nc, ident)

            cc_in = dram.tile([1, N], F32)
            cc_out = dram.tile([1, N], F32)
            r_dram = dram.tile([1, N], F32)

            if o["small_in"]:
                hs_src = dram.tile(hst_shape, BF16)
            else:
                hs_src = hsT_d[:]
            if o["small_out"]:
                out_dst = dram.tile([HID, N], out_dt)
            else:
                out_dst = outp_d[:]

            if not o["hst_contig"]:
                hsT_r = hs_src.rearrange("(kt p) s -> p kt s", p=128)

            ghts = {}

            def load_hst(c):
                C0 = c * CHUNK
                hst_lo = hstp.tile([128, HID // 256, CHUNK], BF16, name="hst_lo", tag="hst_lo")
                hst_hi = hstp.tile([128, HID // 256, CHUNK], BF16, name="hst_hi", tag="hst_hi")
                if o["hst_contig"]:
                    nc.sync.dma_start(out=hst_lo, in_=hs_src[:, c, 0 : HID // 256, :])
                    nc.sync.dma_start(out=hst_hi, in_=hs_src[:, c, HID // 256 :, :])
                else:
                    nc.sync.dma_start(
                        out=hst_lo, in_=hsT_r[:, 0 : HID // 256, C0 : C0 + CHUNK]
                    )
                    nc.sync.dma_start(
                        out=hst_hi, in_=hsT_r[:, HID // 256 :, C0 : C0 + CHUNK]
                    )
                return hst_lo, hst_hi

            # chunk 0 activations first, then weights m-tile by m-tile so the
            # first projection group starts after ~1.5MB of DMA, not 6MB
            hst0 = load_hst(0)
            for mt in range(8):
                nc.sync.dma_start(
                    out=wcm[mt], in_=wc_d[mt].rearrange("p (kt m) -> p kt m", m=128)
                )
            nc.sync.dma_start(out=dd_sb, in_=dd_d[:])
            nc.sync.dma_start(out=qd_sb, in_=qd_d[:])
            nc.sync.dma_start(out=kdb_sb, in_=kdb_d[:])
            nc.sync.dma_start(out=bd_sb, in_=bd_d[:])
            nc.sync.dma_start(out=S32, in_=kv0_d[:])
            for p in range(HL // 2):
                if o["interbd"]:
                    for hi in range(2):
                        b = hi * 64
                        nc.vector.tensor_copy(
                            Sbd[b : b + 64, p, b : b + 64], S32[b : b + 64, p, :]
                        )
                else:
                    nc.vector.tensor_copy(Sbf[:, p, :], S32[:, p, :])

            def start_chunk(c, hst_pre=None):
                hst = hst_pre if hst_pre is not None else load_hst(c)
                comb = combp.tile([128, 6, CHUNK], BF16, name="comb")
                gate = gatep.tile([128, 2, CHUNK], F32, name="gate")
                ght = ghp.tile([128, 2, CHUNK], BF16, name="ght", tag="ght")
                ghts[c] = ght
                return dict(c=c, hst=hst, comb=comb, gate=gate, ght=ght, mt=0)

            def proj_step(st):
                # emit one projection slice; returns False when exhausted.
                # ilv=1: a slice is a full mt group (16 kt matmuls); ilv=2:
                # half a group (8 kt), giving twice the interleave resolution
                if st is None:
                    return False
                mt = st["mt"]
                if mt >= 9:
                    return False
                if mt == 8:
                    nc.vector.tensor_scalar_add(st["gate"][:], st["gate"][:], 1.0)
                    st["mt"] = 9
                    return True
                hst_lo, hst_hi = st["hst"]
                nk = HID // 128
                if o["ilv"] >= 2:
                    half = st.get("half", 0)
                    if half == 0:
                        st["pj"] = psA.tile([128, CHUNK], F32, tag="pj", name="pj")
                    pj = st["pj"]
                    k0, k1 = half * (nk // 2), (half + 1) * (nk // 2)
                else:
                    pj = psA.tile([128, CHUNK], F32, tag="pj", name="pj")
                    k0, k1 = 0, nk
                for kt in range(k0, k1):
                    hsth = hst_lo if kt < HID // 256 else hst_hi
                    nc.tensor.matmul(
                        pj,
                        lhsT=wcm[mt][:, kt, :],
                        rhs=hsth[:, kt % (HID // 256), :],
                        start=(kt == 0),
                        stop=(kt == nk - 1),
                    )
                if o["ilv"] >= 2 and k1 < nk:
                    st["half"] = 1
                    return True
                st["half"] = 0
                if mt < 6:
                    nc.scalar.activation(st["comb"][:, mt, :], pj, AF.Silu)
                else:
                    # sigmoid(x) = 0.5*(1 + tanh(x/2)); the 0.5 is folded
                    # into W_out on the host, the +1 is applied below.
                    nc.scalar.activation(
                        st["gate"][:, mt - 6, :], pj, AF.Tanh, scale=0.5
                    )
                st["mt"] = mt + 1
                return True

            def emit_chunk(c, hst_pre=None, st=None, fill=None):
                C0 = c * CHUNK
                if st is None:
                    st = start_chunk(c, hst_pre)
                    while proj_step(st):
                        pass
                if fill is None:
                    def fill():
                        return None
                comb, gate, ght = st["comb"], st["gate"], st["ght"]

                if o["no_attn"]:
                    # timing-only stand-in: skip attention/variance/collective
                    for t in range(2):
                        nc.vector.tensor_copy(ght[:, t, :], comb[:, t, :])
                    return

                ps_var = psV.tile([1, CHUNK], F32, tag="var", name="ps_var")

                for blk in range(BPC):
                    bc = blk * BLOCK
                    kn = {}
                    vn = {}
                    # phase A: k-decay pre-scale + PE-transpose k', v
                    for p in range(2):
                        kpt = kptp.tile([128, BLOCK], BF16, tag="kpt", name="kpt")
                        ve_a = nc.gpsimd if o["pooloff"] else nc.vector
                        ve_a.tensor_mul(
                            kpt, comb[:, 2 + p, bc : bc + BLOCK], kdb_sb[:, p, :]
                        )
                        knt = kvp.tile([128, 2, 128], BF16, tag="kn", name="knt")
                        vnt = kvp.tile([128, 2, 128], BF16, tag="vn", name="vnt")
                        kn[p] = knt
                        vn[p] = vnt
                        if o["tpcomb"]:
                            tpkv = psB.tile([128, 4, 128], BF16, tag="qk", name="tpkv")
                            tpk = tpkv[:, 0:2, :].rearrange("p a b -> p (a b)")
                            tpv = tpkv[:, 2:4, :].rearrange("p a b -> p (a b)")
                        else:
                            tpk = psB.tile([128, 512], BF16, tag="qk", name="tpk")
                            tpv = psB.tile([128, 512], BF16, tag="qk", name="tpv")
                        for half in range(2):
                            nc.tensor.transpose(
                                tpk[:, half * 128 : (half + 1) * 128],
                                in_=kpt[:, half * 128 : (half + 1) * 128],
                                identity=ident,
                            )
                            nc.tensor.transpose(
                                tpv[:, half * 128 : (half + 1) * 128],
                                in_=comb[:, 4 + p, bc + half * 128 : bc + (half + 1) * 128],
                                identity=ident,
                            )
                        nc.vector.tensor_copy(
                            knt[:].rearrange("p a b -> p (a b)"), tpk[:, 0:256]
                        )
                        nc.scalar.copy(
                            vnt[:].rearrange("p a b -> p (a b)"), tpv[:, 0:256]
                        )
                        if o["ilv"] >= 2 and p == 0:
                            fill()
                    fill()
                    # phase B: decayed queries
                    qps = {}
                    for p in range(2):
                        qp = qpp.tile([128, BLOCK], BF16, tag="qp", name="qp")
                        qps[p] = qp
                        ve_b = nc.gpsimd if o["pooloff"] else nc.vector
                        for hi in range(2):
                            b = hi * 64
                            ve_b.tensor_mul(
                                qp[b : b + 64, :],
                                comb[b : b + 64, p, bc : bc + BLOCK],
                                qd_sb[b : b + 64, p, :],
                            )
                    # phase C: scores (transposed) + decay mask
                    qkp = {}
                    for p in range(2):
                        for hi in range(2):
                            h = 2 * p + hi
                            b = hi * 64
                            qkph = qkpp.tile([128, 2, BLOCK], BF16, tag="qkp", name="qkph")
                            qkp[h] = qkph
                            qk_ps = psB.tile([128, 512], F32, tag="qk", name="qk_ps")
                            for half in range(2):
                                nc.tensor.matmul(
                                    qk_ps[:, half * 256 : (half + 1) * 256],
                                    lhsT=comb[b : b + 64, 2 + p, bc + half * 128 : bc + (half + 1) * 128],
                                    rhs=comb[b : b + 64, p, bc : bc + BLOCK],
                                    start=True,
                                    stop=True,
                                    tile_position=(b, 0),
                                )
                            nc.vector.tensor_mul(
                                qkph[:].rearrange("p a b -> p (a b)"),
                                qk_ps,
                                dd_sb[:, 2 * h : 2 * h + 2, :].rearrange("p a b -> p (a b)"),
                            )
                    fill()
                    # phase D: attention output (transposed): inter + intra
                    pos = {}
                    for p in range(2):
                        po = psC.tile([128, BLOCK], F32, tag="po", name="po")
                        pos[p] = po
                        if o["interbd"]:
                            nc.tensor.matmul(
                                po,
                                lhsT=Sbd[:, p, :],
                                rhs=qps[p],
                                start=True,
                                stop=False,
                            )
                        else:
                            for hi in range(2):
                                b = hi * 64
                                nc.tensor.matmul(
                                    po[b : b + 64, :],
                                    lhsT=Sbf[b : b + 64, p, :],
                                    rhs=qps[p][b : b + 64, :],
                                    start=True,
                                    stop=False,
                                    tile_position=(b, b),
                                )
                        for hi in range(2):
                            b = hi * 64
                            for half in range(2):
                                nc.tensor.matmul(
                                    po[b : b + 64, :],
                                    lhsT=vn[p][:, half, b : b + 64],
                                    rhs=qkp[2 * p + hi][:, half, :],
                                    start=False,
                                    stop=(half == 1),
                                    tile_position=(0, b),
                                )
                        if o["ilv"] >= 2 and p == 0:
                            fill()
                    fill()
                    # phase E: variance partials + gated hidden
                    for p in range(2):
                        hsq = hsqp.tile([128, BLOCK], BF16, tag="hsq", name="hsq")
                        nc.scalar.square(hsq, pos[p])
                        nc.tensor.matmul(
                            ps_var[0:1, bc : bc + BLOCK],
                            lhsT=ones_sb,
                            rhs=hsq,
                            start=(p == 0),
                            stop=(p == 1),
                        )
                        nc.vector.tensor_mul(
                            ght[:, p, bc : bc + BLOCK], pos[p], gate[:, p, bc : bc + BLOCK]
                        )
                    # phase F: state update S = bd*S + k'^T v
                    for p in range(2):
                        if o["statemerge"]:
                            # full-width [128,128] product; only the diagonal
                            # head quadrants are read, the rest is garbage
                            psS = psC.tile([128, 128], F32, tag="po", name="psS")
                            for half in range(2):
                                nc.tensor.matmul(
                                    psS,
                                    lhsT=kn[p][:, half, :],
                                    rhs=vn[p][:, half, :],
                                    start=(half == 0),
                                    stop=(half == 1),
                                )
                            nc.scalar.mul(
                                S32[:, p, :], S32[:, p, :], bd_sb[:, p : p + 1]
                            )
                            for hi in range(2):
                                b = hi * 64
                                nc.vector.tensor_add(
                                    S32[b : b + 64, p, :],
                                    S32[b : b + 64, p, :],
                                    psS[b : b + 64, b : b + 64],
                                )
                        else:
                            psS = psC.tile([128, D], F32, tag="po", name="psS")
                            for hi in range(2):
                                b = hi * 64
                                for half in range(2):
                                    nc.tensor.matmul(
                                        psS[b : b + 64, :],
                                        lhsT=kn[p][:, half, b : b + 64],
                                        rhs=vn[p][:, half, b : b + 64],
                                        start=(half == 0),
                                        stop=(half == 1),
                                        tile_position=(0, b),
                                    )
                            nc.scalar.mul(
                                S32[:, p, :], S32[:, p, :], bd_sb[:, p : p + 1]
                            )
                            nc.vector.tensor_add(S32[:, p, :], S32[:, p, :], psS)
                        if o["interbd"]:
                            for hi in range(2):
                                b = hi * 64
                                nc.vector.tensor_copy(
                                    Sbd[b : b + 64, p, b : b + 64],
                                    S32[b : b + 64, p, :],
                                )
                        else:
                            nc.vector.tensor_copy(Sbf[:, p, :], S32[:, p, :])
                    fill()

                # variance all-reduce + Newton rsqrt (DVE only), every
                # cc_stride chunks covering the accumulated slice
                ssqc = hsqp.tile([1, CHUNK], F32, tag="ssqc", name="ssqc")
                nc.scalar.copy(ssqc, ps_var)
                nc.scalar.dma_start(out=cc_in[0:1, C0 : C0 + CHUNK], in_=ssqc)
                stride = o["cc_stride"]
                if (c + 1) % stride == 0:
                    R0 = (c + 1 - stride) * CHUNK
                    RW = stride * CHUNK
                    if o["no_cc"]:
                        nc.gpsimd.dma_start(
                            out=cc_out[0:1, R0 : R0 + RW],
                            in_=cc_in[0:1, R0 : R0 + RW],
                        )
                    else:
                        nc.gpsimd.collective_compute(
                            "AllReduce",
                            mybir.AluOpType.add,
                            replica_groups=[list(range(NCORES))],
                            ins=[cc_in[0:1, R0 : R0 + RW].opt()],
                            outs=[cc_out[0:1, R0 : R0 + RW].opt()],
                        )
                    if o["rsq_flat"]:
                        zshape = [1, RW]
                        z = rcp.tile(zshape, F32, tag="z", name="z")
                        nc.gpsimd.dma_start(out=z, in_=cc_out[0:1, R0 : R0 + RW])
                    else:
                        zshape = [128, RW // 128]
                        z = rcp.tile(zshape, F32, tag="z", name="z")
                        nc.gpsimd.dma_start(
                            out=z,
                            in_=cc_out[0:1, R0 : R0 + RW].rearrange(
                                "a (p j) -> (a p) j", p=128
                            ),
                        )
                    nc.vector.tensor_scalar(
                        out=z, in0=z, scalar1=1.0 / HID, scalar2=EPS, op0=ALU.mult, op1=ALU.add
                    )
                    yi = rcp.tile(zshape, I32, tag="yi", name="yi")
                    nc.vector.tensor_scalar(
                        out=yi, in0=z.bitcast(I32), scalar1=1, scalar2=None,
                        op0=ALU.logical_shift_right,
                    )
                    nc.vector.tensor_scalar(
                        out=yi, in0=yi, scalar1=-1, scalar2=MAGIC, op0=ALU.mult, op1=ALU.add
                    )
                    y = yi.bitcast(F32)
                    t = rcp.tile(zshape, F32, tag="t", name="t")
                    for _ in range(2):
                        nc.vector.tensor_mul(t, y, y)
                        nc.vector.tensor_mul(t, t, z)
                        nc.vector.tensor_scalar(
                            out=t, in0=t, scalar1=-0.5, scalar2=1.5, op0=ALU.mult, op1=ALU.add
                        )
                        nc.vector.tensor_mul(y, y, t)
                    if o["rsq_flat"]:
                        nc.gpsimd.dma_start(
                            out=r_dram[0:1, R0 : R0 + RW], in_=y
                        )
                    else:
                        nc.gpsimd.dma_start(
                            out=r_dram[0:1, R0 : R0 + RW].rearrange(
                                "a (p j) -> (a p) j", p=128
                            ),
                            in_=y,
                        )

            def emit_outproj(c):
                C0 = c * CHUNK
                ght = ghts.pop(c)
                rbc = osbp.tile([128, CHUNK], F32, tag="rbc", name="rbc", bufs=o["rbcb"])
                nc.sync.dma_start(
                    out=rbc, in_=r_dram[0:1, C0 : C0 + CHUNK].to_broadcast([128, CHUNK])
                )
                ve_o = nc.gpsimd if o["pooloff"] else nc.vector
                for t in range(2):
                    ve_o.tensor_mul(ght[:, t, :], ght[:, t, :], rbc)
                if o["osb_batch"]:
                    OG = 4
                    for mg in range(HID // 128 // OG):
                        osb = osbp.tile(
                            [128, OG, CHUNK], out_dt, tag="osb", name="osb",
                            bufs=max(2, o["osbb"] // OG),
                        )
                        for j in range(OG):
                            mt = mg * OG + j
                            pj = psA.tile([128, CHUNK], F32, tag="pj", name="pjo")
                            for kt in range(2):
                                nc.tensor.matmul(
                                    pj,
                                    lhsT=wo_sb[:, kt, mt * 128 : (mt + 1) * 128],
                                    rhs=ght[:, kt, :],
                                    start=(kt == 0),
                                    stop=(kt == 1),
                                )
                            if mt % 2 == 0:
                                nc.scalar.copy(osb[:, j, :], pj)
                            else:
                                nc.vector.tensor_copy(osb[:, j, :], pj)
                        nc.sync.dma_start(
                            out=out_dst[
                                mg * OG * 128 : (mg + 1) * OG * 128, C0 : C0 + CHUNK
                            ].rearrange("(j p) s -> p j s", p=128),
                            in_=osb,
                        )
                else:
                    for mt in range(HID // 128):
                        pj = psA.tile([128, CHUNK], F32, tag="pj", name="pjo")
                        for kt in range(2):
                            nc.tensor.matmul(
                                pj,
                                lhsT=wo_sb[:, kt, mt * 128 : (mt + 1) * 128],
                                rhs=ght[:, kt, :],
                                start=(kt == 0),
                                stop=(kt == 1),
                            )
                        osb = osbp.tile([128, CHUNK], out_dt, tag="osb", name="osb")
                        if mt % 2 == 0:
                            nc.scalar.copy(osb, pj)
                        else:
                            nc.vector.tensor_copy(osb, pj)
                        nc.sync.dma_start(
                            out=out_dst[mt * 128 : (mt + 1) * 128, C0 : C0 + CHUNK],
                            in_=osb,
                        )

            LAG = o["lag"]
            TAPER = o["taper"]
            wo_loaded = False
            for rep in range(o["rep"]):
                done = 0  # next outproj chunk to emit
                if o["ilv"]:
                    # interleaved emission: chunk c+1's projection slices are
                    # emitted between chunk c's attention phases so the baked
                    # PE order always has independent matmuls behind a
                    # dependency-stalled attention op
                    st_cur = start_chunk(0, hst_pre=hst0 if rep == 0 else None)
                    while proj_step(st_cur):
                        pass
                    for c in range(NCHUNK):
                        st_next = (
                            start_chunk(c + 1) if c + 1 < NCHUNK else None
                        )
                        emit_chunk(c, st=st_cur,
                                   fill=lambda s=st_next: proj_step(s))
                        while proj_step(st_next):
                            pass
                        st_cur = st_next
                        if not wo_loaded:
                            nc.sync.dma_start(
                                out=wo_sb,
                                in_=wo_d[:].rearrange("(kt p) m -> p kt m", p=128),
                            )
                            wo_loaded = True
                        want = c + 1 - LAG
                        if c >= NCHUNK - TAPER:
                            want = c + 1 - max(
                                1, LAG - 2 * (c - (NCHUNK - TAPER)) - 2
                            )
                        while done < min(want, c):
                            emit_outproj(done)
                            done += 1
                    while done < NCHUNK:
                        emit_outproj(done)
                        done += 1
                    continue
                for c in range(NCHUNK):
                    if o["dma_only"]:
                        if c > 0 or rep > 0:
                            load_hst(c)
                        continue
                    emit_chunk(
                        c,
                        hst_pre=hst0
                        if (c == 0 and rep == 0) or o["proj_noload"]
                        else None,
                    )
                    if o["no_outproj"]:
                        ghts.pop(c, None)
                        continue
                    if not wo_loaded:
                        nc.sync.dma_start(
                            out=wo_sb,
                            in_=wo_d[:].rearrange("(kt p) m -> p kt m", p=128),
                        )
                        wo_loaded = True
                    # steady state: keep `done` LAG chunks behind; in the last
                    # TAPER chunks emit two outprojs per chunk to shrink the tail
                    want = c + 1 - LAG
                    if c >= NCHUNK - TAPER:
                        want = c + 1 - max(1, LAG - 2 * (c - (NCHUNK - TAPER)) - 2)
                    while done < min(want, c):
                        emit_outproj(done)
                        done += 1
                while done < NCHUNK and not (o["no_outproj"] or o["dma_only"]):
                    emit_outproj(done)
                    done += 1

    nc.finalize()
    return nc


def _prep_inputs(hidden_states, kv_cache, W_qkv, W_gate, W_out, norm_weight,
                 hst_contig=0):
    if hst_contig:
        # chunk-major layout [p, c, kt, s]: each chunk load is contiguous
        hsT = np.ascontiguousarray(
            hidden_states.T.reshape(HID // 128, 128, NCHUNK, CHUNK).transpose(
                1, 2, 0, 3
            )
        ).astype(BF)
    else:
        hsT = np.ascontiguousarray(hidden_states.T).astype(BF)
    in_maps = []
    arr = np.arange(BLOCK, dtype=np.float32) + 1.0  # 1..256
    nloc = np.arange(BLOCK, dtype=np.float32)
    for c in range(NCORES):
        heads = [4 * c + h for h in range(HL)]
        # fused weight: [Q(4x64), K(4x64), V(4x64), gate(256)] x HID
        rows = []
        for part in range(3):  # q, k, v
            for g in heads:
                base = g * 3 * D + part * D
                rows.append(W_qkv[base : base + D])
        rows.append(W_gate[c * 256 : (c + 1) * 256])
        w_comb = np.concatenate(rows, axis=0)  # [1024, HID]
        # mt-major SBUF image: [mt, p, kt*128+m] with element = W_combT[kt*128+p, mt*128+m]
        wcomb = np.ascontiguousarray(
            w_comb.T.reshape(HID // 128, 128, 8, 128).transpose(2, 1, 0, 3).reshape(8, 128, HID)
        ).astype(BF)

        # 0.5 factor: gate sigmoid computed on-device as tanh-based 1+tanh(x/2)
        w_out_c = (
            W_out[:, c * 256 : (c + 1) * 256]
            * norm_weight[c * 256 : (c + 1) * 256][None, :]
            * 0.5
        )
        wout = np.ascontiguousarray(w_out_c.T).astype(BF)  # [256, HID]

        s = SLOPE[heads]  # [4]
        qd = np.zeros((128, HL // 2, BLOCK), np.float32)
        kdb = np.zeros((128, HL // 2, BLOCK), np.float32)
        dd = np.zeros((128, 2 * HL, BLOCK), np.float32)
        bd = np.zeros((128, HL // 2), np.float32)
        kv0 = np.zeros((128, HL // 2, D), np.float32)
        for h in range(HL):
            sh = s[h]
            b = (h % 2) * 64
            p = h // 2
            qd[b : b + 64, p, :] = np.exp(-sh * arr)[None, :]
            kdb[b : b + 64, p, :] = np.exp(-sh * (BLOCK - nloc - 1))[None, :]
            bd[b : b + 64, p] = math.exp(-sh * BLOCK)
            kv0[b : b + 64, p, :] = kv_cache[heads[h]]
            for half in range(2):
                npos = half * 128 + nloc[:128]
                idx = arr[None, :] - 1 - npos[:, None]  # m - n
                dd[:, 2 * h + half, :] = np.where(idx >= 0, np.exp(-sh * idx), 0.0)
        in_maps.append(
            {
                "hsT": hsT,
                "wcomb": wcomb,
                "wout": wout,
                "dd": dd,
                "qd": qd.astype(BF),
                "kdb": kdb.astype(BF),
                "bd": bd,
                "kv0": kv0,
            }
        )
    return in_maps


# opts used by the graded kernel() entry point
BEST_OPTS = dict(interbd=1, statemerge=1, osb_f16=1, ilv=1)


def kernel(**inputs):
    global _NC_CACHE, LAST_EXEC_NS, LAST_RESULTS
    hidden_states = np.asarray(inputs["hidden_states"], dtype=np.float32)
    kv_cache = np.asarray(inputs["kv_cache"], dtype=np.float32)
    W_qkv = np.asarray(inputs["W_qkv"], dtype=np.float32)
    W_gate = np.asarray(inputs["W_gate"], dtype=np.float32)
    W_out = np.asarray(inputs["W_out"], dtype=np.float32)
    norm_weight = np.asarray(inputs["norm_weight"], dtype=np.float32)

    if _NC_CACHE is None:
        _NC_CACHE = _build_module(**BEST_OPTS)
    nc = _NC_CACHE

    in_maps = _prep_inputs(
        hidden_states, kv_cache, W_qkv, W_gate, W_out, norm_weight,
        hst_contig=BEST_OPTS.get("hst_contig", 0),
    )
    res = run_bass_kernel_spmd(nc, in_maps, core_ids=list(range(NCORES)))
    LAST_EXEC_NS = res.exec_time_ns
    LAST_RESULTS = res
    acc = res.results[0]["outp"].astype(np.float64)
    for c in range(1, NCORES):
        acc += res.results[c]["outp"]
    return np.ascontiguousarray(acc.T).astype(np.float32)

